# revision 4
# baseline (speedup 1.0000x reference)
"""GriddingDistance trilinear scatter kernel for trn2 (8 NeuronCores).

Sharding: data-parallel over batch (8 samples -> 8 cores). Each core
computes the full (G,) voxel grids for its sample's pred and gt clouds.

Per-core algorithm: the 8 trilinear corner weights factor as
wx(sx)*wy(sy)*wz(sz).  For each of the 4 (x,y) corner cells
(q = (x0+sx)*128 + (y0+sy) in [0,16384)) the z-contribution is the
128-wide profile relu(1 - |pz - z|) * wxy, which equals (1-dz) at z0,
dz at z0+1, 0 elsewhere.  The grid lives in DRAM as [16384, 128] rows;
contributions are applied in tiles of 128 rows: PE-transpose +
is_equal selection matrix (accumulates duplicate-q rows), PE matmul to
form per-row full sums, indirect-DMA gather of the 128 grid rows, DVE
add, indirect-DMA scatter back (duplicate rows write identical values).
"""

import numpy as np

P = 128
N_PTS = 65536
NPB = N_PTS // P  # 512 points per partition
R = 128
NQ = R * R  # 16384 xy-cells
G = R * R * R
SCALE = 128.0
GRID_MIN = -64.0

_cache = {}


def _build():
    import concourse.bacc as bacc
    import concourse.mybir as mybir
    import concourse.bass as bass
    from concourse.tile import TileContext
    from concourse.masks import make_identity

    nc = bacc.Bacc(None, target_bir_lowering=False)
    f32 = mybir.dt.float32
    i32 = mybir.dt.int32
    Alu = mybir.AluOpType
    Act = mybir.ActivationFunctionType

    clouds_in = nc.dram_tensor("clouds", [2, P, NPB * 3], f32, kind="ExternalInput")
    grids = [
        nc.dram_tensor(f"grid{c}", [NQ, R], f32, kind="ExternalOutput")
        for c in range(2)
    ]

    with TileContext(nc) as tc:
        with (
            tc.tile_pool(name="const", bufs=1) as cpool,
            tc.tile_pool(name="planes", bufs=1) as ppool,
            tc.tile_pool(name="work", bufs=3) as wpool,
            tc.tile_pool(name="psum", bufs=4, space="PSUM") as pspool,
        ):
            ident = cpool.tile([P, P], f32)
            make_identity(nc, ident[:])
            iotai = cpool.tile([P, R], i32)
            nc.gpsimd.iota(iotai[:], pattern=[[1, R]], base=0, channel_multiplier=0)
            iotaf = cpool.tile([P, R], f32)
            nc.vector.tensor_copy(out=iotaf[:], in_=iotai[:])
            zero_rows = cpool.tile([P, R], f32)
            nc.vector.memset(zero_rows[:], 0.0)

            # zero both output grids
            for c in range(2):
                for blk in range(NQ // P):
                    nc.sync.dma_start(
                        out=grids[c][blk * P : (blk + 1) * P, :], in_=zero_rows[:]
                    )

            # ---- Phase A: per-cloud point math -> persistent planes ----
            PZ, Q, W = [], [], []
            for c in range(2):
                raw = wpool.tile([P, NPB * 3], f32, tag="raw")
                nc.sync.dma_start(out=raw[:], in_=clouds_in[c])
                rv = raw[:].rearrange("p (n t) -> p n t", t=3)
                crd, flo = [], []
                for t in range(3):
                    cc = wpool.tile([P, NPB], f32, tag=f"crd{t}")
                    # p' = cloud*128 + 64, strictly inside (1.2, 126.8)
                    nc.scalar.activation(
                        cc[:], rv[:, :, t], Act.Copy, bias=-GRID_MIN, scale=SCALE
                    )
                    crd.append(cc)
                    if t < 2:
                        # floor: round via i32 convert, then subtract (round > x)
                        fi = wpool.tile([P, NPB], i32, tag=f"fi{t}")
                        ff = wpool.tile([P, NPB], f32, tag=f"ff{t}")
                        gt = wpool.tile([P, NPB], f32, tag=f"gt{t}")
                        nc.vector.tensor_copy(out=fi[:], in_=cc[:])
                        nc.vector.tensor_copy(out=ff[:], in_=fi[:])
                        nc.vector.tensor_tensor(
                            out=gt[:], in0=ff[:], in1=cc[:], op=Alu.is_gt
                        )
                        nc.vector.tensor_tensor(
                            out=ff[:], in0=ff[:], in1=gt[:], op=Alu.subtract
                        )
                        flo.append(ff)
                # fractional parts for x,y
                wx1 = wpool.tile([P, NPB], f32, tag="wx1")
                wy1 = wpool.tile([P, NPB], f32, tag="wy1")
                nc.vector.tensor_tensor(
                    out=wx1[:], in0=crd[0][:], in1=flo[0][:], op=Alu.subtract
                )
                nc.vector.tensor_tensor(
                    out=wy1[:], in0=crd[1][:], in1=flo[1][:], op=Alu.subtract
                )
                wx0 = wpool.tile([P, NPB], f32, tag="wx0")
                wy0 = wpool.tile([P, NPB], f32, tag="wy0")
                nc.vector.tensor_scalar(
                    out=wx0[:], in0=wx1[:], scalar1=-1.0, scalar2=1.0,
                    op0=Alu.mult, op1=Alu.add,
                )
                nc.vector.tensor_scalar(
                    out=wy0[:], in0=wy1[:], scalar1=-1.0, scalar2=1.0,
                    op0=Alu.mult, op1=Alu.add,
                )
                # qbase = x0*128 + y0 (exact in f32)
                qb = wpool.tile([P, NPB], f32, tag="qb")
                nc.vector.tensor_scalar(
                    out=qb[:], in0=flo[0][:], scalar1=float(R), scalar2=None,
                    op0=Alu.mult,
                )
                nc.vector.tensor_tensor(
                    out=qb[:], in0=qb[:], in1=flo[1][:], op=Alu.add
                )
                pzp = ppool.tile([P, NPB], f32, tag=f"PZ{c}")
                nc.vector.tensor_copy(out=pzp[:], in_=crd[2][:])
                PZ.append(pzp)
                Qc, Wc = [], []
                for idx, (sx, sy) in enumerate(((0, 0), (0, 1), (1, 0), (1, 1))):
                    qf = wpool.tile([P, NPB], f32, tag="qtmp")
                    nc.vector.tensor_scalar(
                        out=qf[:], in0=qb[:], scalar1=float(sx * R + sy),
                        scalar2=None, op0=Alu.add,
                    )
                    qp = ppool.tile([P, NPB], i32, tag=f"Q{c}{idx}")
                    nc.vector.tensor_copy(out=qp[:], in_=qf[:])
                    wp = ppool.tile([P, NPB], f32, tag=f"W{c}{idx}")
                    nc.vector.tensor_tensor(
                        out=wp[:],
                        in0=(wx1 if sx else wx0)[:],
                        in1=(wy1 if sy else wy0)[:],
                        op=Alu.mult,
                    )
                    Qc.append(qp)
                    Wc.append(wp)
                Q.append(Qc)
                W.append(Wc)

            # ---- Phase B: scatter, one 128-row tile per (cloud, cell, col) ----
            def tile_unit(c, k, col):
                qcol = Q[c][k][:, col]
                pzcol = PZ[c][:, col]
                wcol = W[c][k][:, col]
                prof = wpool.tile([P, R], f32, tag="prof")
                # t = iota - pz ; prof = relu(1 - |t|) * wxy
                nc.vector.tensor_scalar(
                    out=prof[:], in0=iotaf[:], scalar1=pzcol, scalar2=None,
                    op0=Alu.subtract,
                )
                nc.scalar.activation(prof[:], prof[:], Act.Abs)
                nc.scalar.activation(prof[:], prof[:], Act.Relu, bias=1.0, scale=-1.0)
                nc.vector.tensor_scalar_mul(prof[:], prof[:], wcol)
                # selection matrix for intra-tile duplicate q
                qf = wpool.tile([P, 1], f32, tag="qf1")
                nc.vector.tensor_copy(out=qf[:], in_=qcol)
                qfix = wpool.tile([P, 1], i32, tag="qfix")
                nc.vector.tensor_copy(out=qfix[:], in_=qcol)
                qT_ps = pspool.tile([P, P], f32, tag="qT")
                nc.tensor.transpose(
                    out=qT_ps[:], in_=qf[:].to_broadcast([P, P]), identity=ident[:]
                )
                sel = wpool.tile([P, P], f32, tag="sel")
                nc.vector.tensor_tensor(
                    out=sel[:], in0=qf[:].to_broadcast([P, P]), in1=qT_ps[:],
                    op=Alu.is_equal,
                )
                summed_ps = pspool.tile([P, R], f32, tag="summed")
                nc.tensor.matmul(
                    out=summed_ps[:], lhsT=sel[:], rhs=prof[:], start=True, stop=True
                )
                rows = wpool.tile([P, R], f32, tag="rows")
                nc.gpsimd.indirect_dma_start(
                    out=rows[:], out_offset=None, in_=grids[c][:],
                    in_offset=bass.IndirectOffsetOnAxis(ap=qfix[:, :1], axis=0),
                )
                nc.vector.tensor_tensor(
                    out=rows[:], in0=rows[:], in1=summed_ps[:], op=Alu.add
                )
                nc.gpsimd.indirect_dma_start(
                    out=grids[c][:],
                    out_offset=bass.IndirectOffsetOnAxis(ap=qfix[:, :1], axis=0),
                    in_=rows[:], in_offset=None,
                )

            with tc.For_i(0, NPB, 1) as i:
                col = bass.ds(i, 1)
                for c in range(2):
                    for k in range(4):
                        tile_unit(c, k, col)

    nc.compile()
    return nc


def _get_nc():
    if "nc" not in _cache:
        _cache["nc"] = _build()
    return _cache["nc"]


def kernel(pred_cloud: np.ndarray, gt_cloud: np.ndarray):
    from concourse.bass_utils import run_bass_kernel_spmd

    nc = _get_nc()
    b = pred_cloud.shape[0]
    in_maps = []
    for core in range(8):
        arr = np.stack(
            [
                pred_cloud[core].astype(np.float32).reshape(P, NPB * 3),
                gt_cloud[core].astype(np.float32).reshape(P, NPB * 3),
            ]
        )
        in_maps.append({"clouds": np.ascontiguousarray(arr)})
    res = run_bass_kernel_spmd(nc, in_maps, core_ids=list(range(8)))
    pred_grid = np.stack([res.results[c]["grid0"].reshape(G) for c in range(b)])
    gt_grid = np.stack([res.results[c]["grid1"].reshape(G) for c in range(b)])
    return pred_grid, gt_grid


# revision 7
# speedup vs baseline: 1.0698x; 1.0698x over previous
"""GriddingDistance trilinear scatter kernel for trn2 (8 NeuronCores).

Sharding: data-parallel over batch (8 samples -> 8 cores). Each core
computes the full (G,) voxel grids for its sample's pred and gt clouds.

Per-core algorithm: the 8 trilinear corner weights factor as
wx(sx)*wy(sy)*wz(sz).  For each of the 4 (x,y) corner cells
(q = (x0+sx)*128 + (y0+sy) in [0,16384)) the z-contribution is the
128-wide profile relu(1 - |pz - z|) * wxy, which equals (1-dz) at z0,
dz at z0+1, 0 elsewhere.  The grid lives in DRAM as [16384, 128] rows;
contributions are applied in tiles of 128 rows: PE-transpose +
is_equal selection matrix (accumulates duplicate-q rows), PE matmul to
form per-row full sums, indirect-DMA gather of the 128 grid rows, DVE
add, indirect-DMA scatter back (duplicate rows write identical values).
"""

import numpy as np

P = 128
N_PTS = 65536
NPB = N_PTS // P  # 512 points per partition
R = 128
NQ = R * R  # 16384 xy-cells
G = R * R * R
SCALE = 128.0
GRID_MIN = -64.0

_cache = {}


def _build():
    import concourse.bacc as bacc
    import concourse.mybir as mybir
    import concourse.bass as bass
    from concourse.tile import TileContext
    from concourse.masks import make_identity

    nc = bacc.Bacc(None, target_bir_lowering=False)
    f32 = mybir.dt.float32
    i32 = mybir.dt.int32
    Alu = mybir.AluOpType
    Act = mybir.ActivationFunctionType

    clouds_in = nc.dram_tensor("clouds", [2, P, NPB * 3], f32, kind="ExternalInput")
    grids = [
        nc.dram_tensor(f"grid{c}", [NQ, R], f32, kind="ExternalOutput")
        for c in range(2)
    ]
    # per-(cloud, xy-cell) partial accumulator grids -> 8 independent
    # gather/add/scatter dependency chains that overlap in the DMA queues
    pgrids = [
        [nc.dram_tensor(f"pg{c}_{k}", [NQ, R], f32) for k in range(4)]
        for c in range(2)
    ]

    with TileContext(nc) as tc:
        with (
            tc.tile_pool(name="const", bufs=1) as cpool,
            tc.tile_pool(name="planes", bufs=1) as ppool,
            tc.tile_pool(name="work", bufs=3) as wpool,
            tc.tile_pool(name="psum", bufs=4, space="PSUM") as pspool,
        ):
            ident = cpool.tile([P, P], f32)
            make_identity(nc, ident[:])
            iotai = cpool.tile([P, R], i32)
            nc.gpsimd.iota(iotai[:], pattern=[[1, R]], base=0, channel_multiplier=0)
            iotaf = cpool.tile([P, R], f32)
            nc.vector.tensor_copy(out=iotaf[:], in_=iotai[:])
            zero_rows = cpool.tile([P, R], f32)
            nc.vector.memset(zero_rows[:], 0.0)

            # zero all partial grids
            for c in range(2):
                for k in range(4):
                    for blk in range(NQ // P):
                        nc.sync.dma_start(
                            out=pgrids[c][k][blk * P : (blk + 1) * P, :],
                            in_=zero_rows[:],
                        )

            # ---- Phase A: per-cloud point math -> persistent planes ----
            PZ, Q, W = [], [], []
            for c in range(2):
                raw = wpool.tile([P, NPB * 3], f32, tag="raw")
                nc.sync.dma_start(out=raw[:], in_=clouds_in[c])
                rv = raw[:].rearrange("p (n t) -> p n t", t=3)
                crd, flo = [], []
                for t in range(3):
                    cc = wpool.tile([P, NPB], f32, tag=f"crd{t}")
                    # p' = cloud*128 + 64, strictly inside (1.2, 126.8)
                    nc.scalar.activation(
                        cc[:], rv[:, :, t], Act.Copy, bias=-GRID_MIN, scale=SCALE
                    )
                    crd.append(cc)
                    if t < 2:
                        # floor: round via i32 convert, then subtract (round > x)
                        fi = wpool.tile([P, NPB], i32, tag=f"fi{t}")
                        ff = wpool.tile([P, NPB], f32, tag=f"ff{t}")
                        gt = wpool.tile([P, NPB], f32, tag=f"gt{t}")
                        nc.vector.tensor_copy(out=fi[:], in_=cc[:])
                        nc.vector.tensor_copy(out=ff[:], in_=fi[:])
                        nc.vector.tensor_tensor(
                            out=gt[:], in0=ff[:], in1=cc[:], op=Alu.is_gt
                        )
                        nc.vector.tensor_tensor(
                            out=ff[:], in0=ff[:], in1=gt[:], op=Alu.subtract
                        )
                        flo.append(ff)
                # fractional parts for x,y
                wx1 = wpool.tile([P, NPB], f32, tag="wx1")
                wy1 = wpool.tile([P, NPB], f32, tag="wy1")
                nc.vector.tensor_tensor(
                    out=wx1[:], in0=crd[0][:], in1=flo[0][:], op=Alu.subtract
                )
                nc.vector.tensor_tensor(
                    out=wy1[:], in0=crd[1][:], in1=flo[1][:], op=Alu.subtract
                )
                wx0 = wpool.tile([P, NPB], f32, tag="wx0")
                wy0 = wpool.tile([P, NPB], f32, tag="wy0")
                nc.vector.tensor_scalar(
                    out=wx0[:], in0=wx1[:], scalar1=-1.0, scalar2=1.0,
                    op0=Alu.mult, op1=Alu.add,
                )
                nc.vector.tensor_scalar(
                    out=wy0[:], in0=wy1[:], scalar1=-1.0, scalar2=1.0,
                    op0=Alu.mult, op1=Alu.add,
                )
                # qbase = x0*128 + y0 (exact in f32)
                qb = wpool.tile([P, NPB], f32, tag="qb")
                nc.vector.tensor_scalar(
                    out=qb[:], in0=flo[0][:], scalar1=float(R), scalar2=None,
                    op0=Alu.mult,
                )
                nc.vector.tensor_tensor(
                    out=qb[:], in0=qb[:], in1=flo[1][:], op=Alu.add
                )
                pzp = ppool.tile([P, NPB], f32, tag=f"PZ{c}")
                nc.vector.tensor_copy(out=pzp[:], in_=crd[2][:])
                PZ.append(pzp)
                Qc, Wc = [], []
                for idx, (sx, sy) in enumerate(((0, 0), (0, 1), (1, 0), (1, 1))):
                    qf = wpool.tile([P, NPB], f32, tag="qtmp")
                    nc.vector.tensor_scalar(
                        out=qf[:], in0=qb[:], scalar1=float(sx * R + sy),
                        scalar2=None, op0=Alu.add,
                    )
                    qp = ppool.tile([P, NPB], i32, tag=f"Q{c}{idx}")
                    nc.vector.tensor_copy(out=qp[:], in_=qf[:])
                    wp = ppool.tile([P, NPB], f32, tag=f"W{c}{idx}")
                    nc.vector.tensor_tensor(
                        out=wp[:],
                        in0=(wx1 if sx else wx0)[:],
                        in1=(wy1 if sy else wy0)[:],
                        op=Alu.mult,
                    )
                    Qc.append(qp)
                    Wc.append(wp)
                Q.append(Qc)
                W.append(Wc)

            # ---- Phase B: scatter, one 128-row tile per (cloud, cell, col) ----
            def tile_unit(c, k, col):
                qcol = Q[c][k][:, col]
                pzcol = PZ[c][:, col]
                wcol = W[c][k][:, col]
                prof = wpool.tile([P, R], f32, tag="prof")
                # t = iota - pz ; prof = relu(1 - |t|) * wxy
                nc.vector.tensor_scalar(
                    out=prof[:], in0=iotaf[:], scalar1=pzcol, scalar2=None,
                    op0=Alu.subtract,
                )
                nc.scalar.activation(prof[:], prof[:], Act.Abs)
                nc.scalar.activation(prof[:], prof[:], Act.Relu, bias=1.0, scale=-1.0)
                nc.vector.tensor_scalar_mul(prof[:], prof[:], wcol)
                # selection matrix for intra-tile duplicate q
                qf = wpool.tile([P, 1], f32, tag="qf1")
                nc.vector.tensor_copy(out=qf[:], in_=qcol)
                qfix = wpool.tile([P, 1], i32, tag="qfix")
                nc.vector.tensor_copy(out=qfix[:], in_=qcol)
                qT_ps = pspool.tile([P, P], f32, tag="qT")
                nc.tensor.transpose(
                    out=qT_ps[:], in_=qf[:].to_broadcast([P, P]), identity=ident[:]
                )
                sel = wpool.tile([P, P], f32, tag="sel")
                nc.vector.tensor_tensor(
                    out=sel[:], in0=qf[:].to_broadcast([P, P]), in1=qT_ps[:],
                    op=Alu.is_equal,
                )
                summed_ps = pspool.tile([P, R], f32, tag="summed")
                nc.tensor.matmul(
                    out=summed_ps[:], lhsT=sel[:], rhs=prof[:], start=True, stop=True
                )
                rows = wpool.tile([P, R], f32, tag=f"rows{c}{k}")
                nc.gpsimd.indirect_dma_start(
                    out=rows[:], out_offset=None, in_=pgrids[c][k][:],
                    in_offset=bass.IndirectOffsetOnAxis(ap=qfix[:, :1], axis=0),
                )
                nc.vector.tensor_tensor(
                    out=rows[:], in0=rows[:], in1=summed_ps[:], op=Alu.add
                )
                nc.gpsimd.indirect_dma_start(
                    out=pgrids[c][k][:],
                    out_offset=bass.IndirectOffsetOnAxis(ap=qfix[:, :1], axis=0),
                    in_=rows[:], in_offset=None,
                )

            with tc.For_i(0, NPB, 1) as i:
                col = bass.ds(i, 1)
                for c in range(2):
                    for k in range(4):
                        tile_unit(c, k, col)

            # ---- merge the 4 partial grids per cloud ----
            for c in range(2):
                for blk in range(NQ // P):
                    acc = wpool.tile([P, R], f32, tag="macc")
                    nc.sync.dma_start(
                        out=acc[:], in_=pgrids[c][0][blk * P : (blk + 1) * P, :]
                    )
                    for k in range(1, 4):
                        part = wpool.tile([P, R], f32, tag=f"mp{k}")
                        nc.sync.dma_start(
                            out=part[:],
                            in_=pgrids[c][k][blk * P : (blk + 1) * P, :],
                        )
                        nc.vector.tensor_tensor(
                            out=acc[:], in0=acc[:], in1=part[:], op=Alu.add
                        )
                    nc.sync.dma_start(
                        out=grids[c][blk * P : (blk + 1) * P, :], in_=acc[:]
                    )

    nc.compile()
    return nc


def _get_nc():
    if "nc" not in _cache:
        _cache["nc"] = _build()
    return _cache["nc"]


def kernel(pred_cloud: np.ndarray, gt_cloud: np.ndarray):
    from concourse.bass_utils import run_bass_kernel_spmd

    nc = _get_nc()
    b = pred_cloud.shape[0]
    in_maps = []
    for core in range(8):
        arr = np.stack(
            [
                pred_cloud[core].astype(np.float32).reshape(P, NPB * 3),
                gt_cloud[core].astype(np.float32).reshape(P, NPB * 3),
            ]
        )
        in_maps.append({"clouds": np.ascontiguousarray(arr)})
    res = run_bass_kernel_spmd(nc, in_maps, core_ids=list(range(8)))
    pred_grid = np.stack([res.results[c]["grid0"].reshape(G) for c in range(b)])
    gt_grid = np.stack([res.results[c]["grid1"].reshape(G) for c in range(b)])
    return pred_grid, gt_grid
